# revision 32
# baseline (speedup 1.0000x reference)
"""Trainium2 Bass kernel for nn_DenseGINEConv (GNN message passing).

  out = MLP_u((1+eps)*x + segsum_dst(MLP_e(x[src] + edge_attr)))

Strategy (8 NeuronCores, nodes sharded by dst, 6250/core), v2
("all-matmul segment sum"):

- Nodes are sorted by degree (desc) per core; edges are packed in *layer*
  order per 512-node slice: layer j holds edge j of every node with more
  than j edges.  Because nodes are degree-sorted, layer j occupies node
  columns [0, m_j) -- a contiguous, aligned run.  Each node's slot count is
  padded to even, so layers come in equal-width pairs.  Total stream is
  ~79k slots/core vs 114.7k for 16-slot group padding.
- Edge MLP layer 2 and update MLP layer 1 are composed host-side
  (W21 = We2 @ Wu1), so the per-edge pipeline is:
     h = GELU(We1^T xg + be1)           (tensor + scalar)
     hs = h[layer 2t] + h[layer 2t+1]   (vector, bf16 2x mode)
     psum_slice += W21^T @ hs[...]      (tensor; PSUM accumulation IS the
                                         segment sum -- no scatter, no
                                         tensor_reduce anywhere)
- All node-constant terms are folded host-side into one tensor:
     xc = (1+eps)x + deg*be2 - padtot*(GELU(be1)@We2)
  consumed by a single Wu1 matmul into the same PSUM accumulation.
  Then: y1 = GELU(psum + bu1); out = Wu2^T y1 + bu2.
- The per-core layer widths are maxed across cores so one Bass program
  (compiled at first kernel() call, from the actual input's degree
  profile) serves all 8 cores SPMD; narrower cores ride zero-pad columns
  whose GELU(be1) contribution is exactly corrected via padtot.
"""

import math
import os
import sys
from contextlib import ExitStack

import numpy as np
import ml_dtypes

# concourse (Bass) lives in the trn_rl repo; make kernel.py self-contained
for _p in ("/opt/trn_rl_repo",):
    if os.path.isdir(_p) and _p not in sys.path:
        sys.path.insert(0, _p)

# ---------------------------------------------------------------- constants
N = 50000
E = 600000
D = 128
NC = 8
NPC = N // NC                 # 6250 nodes/core
SL = 512                      # node-slice width (PSUM bank)
NSLICE = (NPC + SL - 1) // SL
GTILE = 1536                  # edge GELU tile width (3 PSUM banks)

BF16 = ml_dtypes.bfloat16


def _gelu(z):
    z = np.asarray(z, dtype=np.float64)
    return 0.5 * z * (1.0 + np.vectorize(math.erf)(z / math.sqrt(2.0)))


def _bf16(a):
    return np.asarray(a).astype(BF16)


# ---------------------------------------------------------------- host plan
def _build_plans(edge_index):
    """Returns (shared, per_core).

    shared: M[i] = layer widths (even count, non-increasing) per slice,
            loff[i] = col offset of each layer inside the slice chunk,
            choff[i] = chunk offset in the stream, stream = total cols.
    per_core: col_of (node->column), deg, padtot (per column), and the
            stream slot index of every edge (by original edge id).
    """
    src = np.asarray(edge_index[0]).astype(np.int64)
    dst = np.asarray(edge_index[1]).astype(np.int64)

    core_of = dst // NPC
    dst_local = dst - core_of * NPC

    per_core = []
    pads_all = []
    for c in range(NC):
        msk = core_of == c
        cloc = dst_local[msk]
        deg = np.bincount(cloc, minlength=NPC).astype(np.int64)
        order = np.argsort(-deg, kind="stable")      # node ids by deg desc
        col_of = np.empty(NPC, dtype=np.int64)
        col_of[order] = np.arange(NPC)
        per_core.append(dict(col_of=col_of, deg=deg, order=order,
                             eid=np.nonzero(msk)[0], cloc=cloc))
        pads_all.append(deg[order])                  # exact slots per column

    # shared layer caps per slice: M_j(i) = max over cores of
    # #{cols in slice with padded slots > j}
    M, loff, choff, = [], [], []
    off = 0
    for i in range(NSLICE):
        lo, hi = i * SL, min((i + 1) * SL, NPC)
        caps = None
        for c in range(NC):
            p = pads_all[c][lo:hi]
            pm = int(p.max()) if len(p) else 0
            cnt = np.bincount(p, minlength=pm + 1)
            mj = len(p) - np.cumsum(cnt)[:-1]        # m_j = #{p > j}, j=0..pm-1
            mj = mj[mj > 0]
            if caps is None:
                caps = mj
            else:
                n = max(len(caps), len(mj))
                a = np.zeros(n, dtype=np.int64); a[:len(caps)] = caps
                b = np.zeros(n, dtype=np.int64); b[:len(mj)] = mj
                caps = np.maximum(a, b)
        if caps is None or len(caps) == 0:
            caps = np.zeros(0, dtype=np.int64)
        M.append(caps)
        lo_ = np.zeros(len(caps) + 1, dtype=np.int64)
        np.cumsum(caps, out=lo_[1:])
        loff.append(lo_)
        choff.append(off)
        off += int(lo_[-1])
    stream = off

    # per-core: slot index for each edge + padtot per column
    for c in range(NC):
        pc = per_core[c]
        cloc = pc["cloc"]
        col = pc["col_of"][cloc]                     # column of each edge
        sl = col // SL
        pos = col - sl * SL
        # rank of edge within its node (edges arbitrary order)
        o = np.argsort(col, kind="stable")
        col_s = col[o]
        starts = np.searchsorted(col_s, np.arange(NPC))
        rank = np.empty(len(col), dtype=np.int64)
        rank[o] = np.arange(len(col)) - starts[col_s]
        # chunk offset + layer offset + pos
        ch = np.asarray(choff, dtype=np.int64)[sl]
        lof = np.empty(len(col), dtype=np.int64)
        for i in range(NSLICE):
            m = sl == i
            if m.any():
                lof[m] = loff[i][rank[m]]
        slot = ch + lof + pos
        pc["slot"] = slot

        # padtot per column: #layers covering the column minus deg
        padtot = np.zeros(NPC, dtype=np.int64)
        for i in range(NSLICE):
            lo, hi = i * SL, min((i + 1) * SL, NPC)
            w = hi - lo
            p = np.arange(w)
            cover = (M[i][None, :] > p[:, None]).sum(axis=1)
            padtot[lo:hi] = cover
        padtot -= pc["deg"][pc["order"]]
        assert padtot.min() >= 0
        pc["padtot"] = padtot
    shared = dict(M=M, loff=loff, choff=choff, stream=stream)
    return shared, per_core


# ---------------------------------------------------------------- bass build
def _build_bass(shared):
    import concourse.mybir as mybir
    from concourse import bacc
    from concourse._compat import get_trn_type
    from concourse.tile import TileContext

    fp32 = mybir.dt.float32
    bf16 = mybir.dt.bfloat16
    AF = mybir.ActivationFunctionType
    Alu = mybir.AluOpType

    STREAM = shared["stream"]
    M = shared["M"]
    loff = shared["loff"]
    choff = shared["choff"]
    LMAX = max(int(l[-1]) for l in loff)
    LMAX = (LMAX + 511) // 512 * 512

    nc = bacc.Bacc(get_trn_type() or "TRN2")

    din = {}
    for name, shape, dt in [
        ("stream", [D, STREAM], bf16),
        ("xcT", [D, NPC], bf16),
        ("We1", [D, D], bf16),
        ("W21", [D, D], bf16),
        ("Wu1", [D, D], bf16),
        ("Wu2", [D, D], bf16),
        ("be1", [D, 1], fp32),
        ("bu1", [D, 1], fp32),
        ("bu2", [D, 1], fp32),
    ]:
        din[name] = nc.declare_dram_parameter(name, shape, dt, isOutput=False)
    outT = nc.declare_dram_parameter("outT", [D, NPC], bf16, isOutput=True)

    with TileContext(nc) as tc, ExitStack() as ctx:
        consts = ctx.enter_context(tc.tile_pool(name="consts", bufs=1))
        xgp = ctx.enter_context(tc.tile_pool(name="xg", bufs=12))
        hp = ctx.enter_context(tc.tile_pool(name="h", bufs=3))
        hsp = ctx.enter_context(tc.tile_pool(name="hs", bufs=3))
        updp = ctx.enter_context(tc.tile_pool(name="upd", bufs=4))
        xcp = ctx.enter_context(tc.tile_pool(name="xc", bufs=3))
        pse = ctx.enter_context(tc.tile_pool(name="pse", bufs=2, space="PSUM"))
        pagg = ctx.enter_context(tc.tile_pool(name="pagg", bufs=2, space="PSUM"))

        def load(name, shape, dt):
            t = consts.tile(shape, dt, tag=name)
            nc.sync.dma_start(out=t[:, :], in_=din[name][:, :])
            return t

        # warm the Gelu activation table while DMAs fill (saves ~1.3us on
        # the scalar critical path)
        warm = consts.tile([D, 1], fp32, tag="warm")
        nc.vector.memset(warm[:, :], 0.0)
        nc.scalar.activation(warm[:, :], warm[:, :], AF.Gelu, bias=0.0)

        We1 = load("We1", [D, D], bf16)
        be1 = load("be1", [D, 1], fp32)
        late = {name: load(name, shape, dt)
                for name, shape, dt in [("W21", [D, D], bf16),
                                        ("Wu1", [D, D], bf16),
                                        ("Wu2", [D, D], bf16),
                                        ("bu1", [D, 1], fp32),
                                        ("bu2", [D, 1], fp32)]}

        for i in range(NSLICE):
            lo = i * SL
            sw = min(SL, NPC - lo)
            L = int(loff[i][-1])
            caps = M[i]

            h = hp.tile([D, LMAX], bf16, tag="h")
            nt = (L + GTILE - 1) // GTILE
            for t in range(nt):
                t0 = t * GTILE
                w = min(GTILE, L - t0)
                xg = xgp.tile([D, GTILE], bf16, tag="xg")
                nc.sync.dma_start(
                    out=xg[:, :w],
                    in_=din["stream"][:, choff[i] + t0:choff[i] + t0 + w])
                ps = pse.tile([D, GTILE], fp32, tag="pse")
                for q0 in range(0, w, 512):
                    q1 = min(q0 + 512, w)
                    nc.tensor.matmul(ps[:, q0:q1], We1[:, :],
                                     xg[:, q0:q1],
                                     start=True, stop=True)
                nc.scalar.activation(h[:, t0:t0 + w], ps[:, :w], AF.Gelu,
                                     bias=be1[:, :])

            # pair layers for one bf16 add round (2x DVE mode); the excess of
            # the wider layer and an odd tail layer feed W21 matmuls directly
            hs = hsp.tile([D, LMAX // 2], bf16, tag="hs")
            hoff = 0
            pruns = []   # (src_is_hs, col offset, width, psum col offset)
            nl = len(caps)
            t = 0
            while t + 1 < nl:
                W = int(caps[t + 1])
                o0, o1 = int(loff[i][t]), int(loff[i][t + 1])
                nc.vector.tensor_tensor(out=hs[:, hoff:hoff + W],
                                        in0=h[:, o0:o0 + W],
                                        in1=h[:, o1:o1 + W], op=Alu.add)
                pruns.append((True, hoff, W, 0))
                hoff += W
                if int(caps[t]) > W:
                    pruns.append((False, o0 + W, int(caps[t]) - W, W))
                t += 2
            if t < nl:
                pruns.append((False, int(loff[i][t]), int(caps[t]), 0))

            xc = xcp.tile([D, SL], bf16, tag="xc")
            nc.sync.dma_start(out=xc[:, :sw], in_=din["xcT"][:, lo:lo + sw])
            pa = pagg.tile([D, SL], fp32, tag="pu")
            nc.tensor.matmul(pa[:, :sw], late["Wu1"][:, :], xc[:, :sw],
                             start=True, stop=(not pruns))
            for k, (is_hs, ho, W, po_) in enumerate(pruns):
                srct = hs if is_hs else h
                nc.tensor.matmul(pa[:, po_:po_ + W], late["W21"][:, :],
                                 srct[:, ho:ho + W],
                                 start=False, stop=(k == len(pruns) - 1),
                                 skip_group_check=True)

            y1 = updp.tile([D, SL], bf16, tag="y1")
            nc.scalar.activation(y1[:, :sw], pa[:, :sw], AF.Gelu,
                                 bias=late["bu1"][:, :])
            po = pagg.tile([D, SL], fp32, tag="pu")
            nc.tensor.matmul(po[:, :sw], late["Wu2"][:, :], y1[:, :sw],
                             start=True, stop=True)
            ot = updp.tile([D, SL], bf16, tag="ot")
            with nc.allow_low_precision("bf16 output"):
                nc.vector.tensor_scalar_add(ot[:, :sw], po[:, :sw],
                                            late["bu2"][:, :])
            nc.sync.dma_start(out=outT[:, lo:lo + sw], in_=ot[:, :sw])

    nc.compile()
    return nc


# ---------------------------------------------------------------- runner
_CACHE = {}


def _in_maps(inputs, shared, per_core):
    x = np.asarray(inputs["x"], dtype=np.float32)
    edge_attr = np.asarray(inputs["edge_attr"], dtype=np.float32)
    src = np.asarray(inputs["edge_index"][0]).astype(np.int64)
    eps = float(np.asarray(inputs["eps"]).reshape(-1)[0])
    be1 = np.asarray(inputs["be1"], dtype=np.float32)
    be2 = np.asarray(inputs["be2"], dtype=np.float32)

    We1b = _bf16(inputs["We1"]).astype(np.float32)
    We2b = _bf16(inputs["We2"]).astype(np.float32)
    Wu1b = _bf16(inputs["Wu1"]).astype(np.float32)
    Wu2b = _bf16(inputs["Wu2"]).astype(np.float32)
    W21 = _bf16(We2b @ Wu1b)
    qW2 = (_gelu(be1).astype(np.float32) @ We2b).astype(np.float32)

    shared_map = {
        "We1": _bf16(inputs["We1"]),
        "W21": W21,
        "Wu1": _bf16(inputs["Wu1"]),
        "Wu2": _bf16(inputs["Wu2"]),
        "be1": be1.reshape(D, 1),
        "bu1": np.asarray(inputs["bu1"], dtype=np.float32).reshape(D, 1),
        "bu2": np.asarray(inputs["bu2"], dtype=np.float32).reshape(D, 1),
    }

    STREAM = shared["stream"]
    maps = []
    for c in range(NC):
        pc = per_core[c]
        combT = np.zeros((D, STREAM), dtype=BF16)
        eid = pc["eid"]
        combT[:, pc["slot"]] = _bf16(x[src[eid]] + edge_attr[eid]).T

        xn = x[c * NPC:(c + 1) * NPC][pc["order"]]   # node features, col order
        degc = pc["deg"][pc["order"]].astype(np.float32)
        xc = ((1.0 + eps) * xn
              + degc[:, None] * be2[None, :]
              - pc["padtot"].astype(np.float32)[:, None] * qW2[None, :])
        m = dict(shared_map)
        m.update(stream=combT, xcT=_bf16(xc.T))
        maps.append(m)
    return maps


def kernel(**inputs):
    from concourse.bass_utils import run_bass_kernel_spmd

    shared, per_core = _build_plans(inputs["edge_index"])
    key = tuple(int(l[-1]) for l in shared["loff"]) + (shared["stream"],)
    if _CACHE.get("key") != key:
        _CACHE["nc"] = _build_bass(shared)
        _CACHE["key"] = key
    nc = _CACHE["nc"]
    maps = _in_maps(inputs, shared, per_core)
    res = run_bass_kernel_spmd(nc, maps, core_ids=list(range(NC)))
    _CACHE["last_results"] = res
    out = np.zeros((N, D), dtype=np.float32)
    for c in range(NC):
        col_of = per_core[c]["col_of"]
        out[c * NPC:(c + 1) * NPC] = \
            res.results[c]["outT"].astype(np.float32)[:, col_of].T
    return out


# revision 33
# speedup vs baseline: 1.0152x; 1.0152x over previous
"""Trainium2 Bass kernel for nn_DenseGINEConv (GNN message passing).

  out = MLP_u((1+eps)*x + segsum_dst(MLP_e(x[src] + edge_attr)))

Strategy (8 NeuronCores, nodes sharded by dst, 6250/core), v2
("all-matmul segment sum"):

- Nodes are sorted by degree (desc) per core; edges are packed in *layer*
  order per 512-node slice: layer j holds edge j of every node with more
  than j edges.  Because nodes are degree-sorted, layer j occupies node
  columns [0, m_j) -- a contiguous, aligned run.  Each node's slot count is
  padded to even, so layers come in equal-width pairs.  Total stream is
  ~79k slots/core vs 114.7k for 16-slot group padding.
- Edge MLP layer 2 and update MLP layer 1 are composed host-side
  (W21 = We2 @ Wu1), so the per-edge pipeline is:
     h = GELU(We1^T xg + be1)           (tensor + scalar)
     hs = h[layer 2t] + h[layer 2t+1]   (vector, bf16 2x mode)
     psum_slice += W21^T @ hs[...]      (tensor; PSUM accumulation IS the
                                         segment sum -- no scatter, no
                                         tensor_reduce anywhere)
- All node-constant terms are folded host-side into one tensor:
     xc = (1+eps)x + deg*be2 - padtot*(GELU(be1)@We2)
  consumed by a single Wu1 matmul into the same PSUM accumulation.
  Then: y1 = GELU(psum + bu1); out = Wu2^T y1 + bu2.
- The per-core layer widths are maxed across cores so one Bass program
  (compiled at first kernel() call, from the actual input's degree
  profile) serves all 8 cores SPMD; narrower cores ride zero-pad columns
  whose GELU(be1) contribution is exactly corrected via padtot.
"""

import math
import os
import sys
from contextlib import ExitStack

import numpy as np
import ml_dtypes

# concourse (Bass) lives in the trn_rl repo; make kernel.py self-contained
for _p in ("/opt/trn_rl_repo",):
    if os.path.isdir(_p) and _p not in sys.path:
        sys.path.insert(0, _p)

# ---------------------------------------------------------------- constants
N = 50000
E = 600000
D = 128
NC = 8
NPC = N // NC                 # 6250 nodes/core
SL = 512                      # node-slice width (PSUM bank)
NSLICE = (NPC + SL - 1) // SL
GTILE = 1536                  # edge GELU tile width (3 PSUM banks)

BF16 = ml_dtypes.bfloat16


def _gelu(z):
    z = np.asarray(z, dtype=np.float64)
    return 0.5 * z * (1.0 + np.vectorize(math.erf)(z / math.sqrt(2.0)))


def _bf16(a):
    return np.asarray(a).astype(BF16)


# ---------------------------------------------------------------- host plan
def _build_plans(edge_index):
    """Returns (shared, per_core).

    shared: M[i] = layer widths (even count, non-increasing) per slice,
            loff[i] = col offset of each layer inside the slice chunk,
            choff[i] = chunk offset in the stream, stream = total cols.
    per_core: col_of (node->column), deg, padtot (per column), and the
            stream slot index of every edge (by original edge id).
    """
    src = np.asarray(edge_index[0]).astype(np.int64)
    dst = np.asarray(edge_index[1]).astype(np.int64)

    core_of = dst // NPC
    dst_local = dst - core_of * NPC

    per_core = []
    pads_all = []
    for c in range(NC):
        msk = core_of == c
        cloc = dst_local[msk]
        deg = np.bincount(cloc, minlength=NPC).astype(np.int64)
        order = np.argsort(-deg, kind="stable")      # node ids by deg desc
        col_of = np.empty(NPC, dtype=np.int64)
        col_of[order] = np.arange(NPC)
        per_core.append(dict(col_of=col_of, deg=deg, order=order,
                             eid=np.nonzero(msk)[0], cloc=cloc))
        pads_all.append(deg[order])                  # exact slots per column

    # shared layer caps per slice: M_j(i) = max over cores of
    # #{cols in slice with padded slots > j}
    M, loff, choff, = [], [], []
    off = 0
    for i in range(NSLICE):
        lo, hi = i * SL, min((i + 1) * SL, NPC)
        caps = None
        for c in range(NC):
            p = pads_all[c][lo:hi]
            pm = int(p.max()) if len(p) else 0
            cnt = np.bincount(p, minlength=pm + 1)
            mj = len(p) - np.cumsum(cnt)[:-1]        # m_j = #{p > j}, j=0..pm-1
            mj = mj[mj > 0]
            if caps is None:
                caps = mj
            else:
                n = max(len(caps), len(mj))
                a = np.zeros(n, dtype=np.int64); a[:len(caps)] = caps
                b = np.zeros(n, dtype=np.int64); b[:len(mj)] = mj
                caps = np.maximum(a, b)
        if caps is None or len(caps) == 0:
            caps = np.zeros(0, dtype=np.int64)
        M.append(caps)
        lo_ = np.zeros(len(caps) + 1, dtype=np.int64)
        np.cumsum(caps, out=lo_[1:])
        loff.append(lo_)
        choff.append(off)
        off += int(lo_[-1])
    stream = off

    # per-core: slot index for each edge + padtot per column
    for c in range(NC):
        pc = per_core[c]
        cloc = pc["cloc"]
        col = pc["col_of"][cloc]                     # column of each edge
        sl = col // SL
        pos = col - sl * SL
        # rank of edge within its node (edges arbitrary order)
        o = np.argsort(col, kind="stable")
        col_s = col[o]
        starts = np.searchsorted(col_s, np.arange(NPC))
        rank = np.empty(len(col), dtype=np.int64)
        rank[o] = np.arange(len(col)) - starts[col_s]
        # chunk offset + layer offset + pos
        ch = np.asarray(choff, dtype=np.int64)[sl]
        lof = np.empty(len(col), dtype=np.int64)
        for i in range(NSLICE):
            m = sl == i
            if m.any():
                lof[m] = loff[i][rank[m]]
        slot = ch + lof + pos
        pc["slot"] = slot

        # padtot per column: #layers covering the column minus deg
        padtot = np.zeros(NPC, dtype=np.int64)
        for i in range(NSLICE):
            lo, hi = i * SL, min((i + 1) * SL, NPC)
            w = hi - lo
            p = np.arange(w)
            cover = (M[i][None, :] > p[:, None]).sum(axis=1)
            padtot[lo:hi] = cover
        padtot -= pc["deg"][pc["order"]]
        assert padtot.min() >= 0
        pc["padtot"] = padtot
    shared = dict(M=M, loff=loff, choff=choff, stream=stream)
    return shared, per_core


# ---------------------------------------------------------------- bass build
def _build_bass(shared):
    import concourse.mybir as mybir
    from concourse import bacc
    from concourse._compat import get_trn_type
    from concourse.tile import TileContext

    fp32 = mybir.dt.float32
    bf16 = mybir.dt.bfloat16
    AF = mybir.ActivationFunctionType
    Alu = mybir.AluOpType

    STREAM = shared["stream"]
    M = shared["M"]
    loff = shared["loff"]
    choff = shared["choff"]
    LMAX = max(int(l[-1]) for l in loff)
    LMAX = (LMAX + 511) // 512 * 512

    nc = bacc.Bacc(get_trn_type() or "TRN2")

    din = {}
    for name, shape, dt in [
        ("stream", [D, STREAM], bf16),
        ("xcT", [D, NPC], bf16),
        ("We1", [D, D], bf16),
        ("W21", [D, D], bf16),
        ("Wu1", [D, D], bf16),
        ("Wu2", [D, D], bf16),
        ("be1", [D, 1], fp32),
        ("bu1", [D, 1], fp32),
        ("bu2", [D, 1], fp32),
    ]:
        din[name] = nc.declare_dram_parameter(name, shape, dt, isOutput=False)
    outT = nc.declare_dram_parameter("outT", [D, NPC], bf16, isOutput=True)

    with TileContext(nc) as tc, ExitStack() as ctx:
        consts = ctx.enter_context(tc.tile_pool(name="consts", bufs=1))
        xgp = ctx.enter_context(tc.tile_pool(name="xg", bufs=12))
        hp = ctx.enter_context(tc.tile_pool(name="h", bufs=3))
        hsp = ctx.enter_context(tc.tile_pool(name="hs", bufs=3))
        updp = ctx.enter_context(tc.tile_pool(name="upd", bufs=4))
        xcp = ctx.enter_context(tc.tile_pool(name="xc", bufs=3))
        pse = ctx.enter_context(tc.tile_pool(name="pse", bufs=2, space="PSUM"))
        pagg = ctx.enter_context(tc.tile_pool(name="pagg", bufs=2, space="PSUM"))

        def load(name, shape, dt):
            # consts issue from the (startup-idle) scalar HWDGE queue so the
            # sync queue starts streaming edge units immediately
            t = consts.tile(shape, dt, tag=name)
            nc.scalar.dma_start(out=t[:, :], in_=din[name][:, :])
            return t

        # warm the Gelu activation table while DMAs fill (saves ~1.3us on
        # the scalar critical path)
        warm = consts.tile([D, 1], fp32, tag="warm")
        nc.vector.memset(warm[:, :], 0.0)
        nc.scalar.activation(warm[:, :], warm[:, :], AF.Gelu, bias=0.0)

        We1 = load("We1", [D, D], bf16)
        be1 = load("be1", [D, 1], fp32)
        late = {name: load(name, shape, dt)
                for name, shape, dt in [("W21", [D, D], bf16),
                                        ("Wu1", [D, D], bf16),
                                        ("Wu2", [D, D], bf16),
                                        ("bu1", [D, 1], fp32),
                                        ("bu2", [D, 1], fp32)]}

        for i in range(NSLICE):
            lo = i * SL
            sw = min(SL, NPC - lo)
            L = int(loff[i][-1])
            caps = M[i]

            h = hp.tile([D, LMAX], bf16, tag="h")
            nt = (L + GTILE - 1) // GTILE
            for t in range(nt):
                t0 = t * GTILE
                w = min(GTILE, L - t0)
                xg = xgp.tile([D, GTILE], bf16, tag="xg")
                nc.sync.dma_start(
                    out=xg[:, :w],
                    in_=din["stream"][:, choff[i] + t0:choff[i] + t0 + w])
                ps = pse.tile([D, GTILE], fp32, tag="pse")
                for q0 in range(0, w, 512):
                    q1 = min(q0 + 512, w)
                    nc.tensor.matmul(ps[:, q0:q1], We1[:, :],
                                     xg[:, q0:q1],
                                     start=True, stop=True)
                nc.scalar.activation(h[:, t0:t0 + w], ps[:, :w], AF.Gelu,
                                     bias=be1[:, :])

            # pair layers for one bf16 add round (2x DVE mode); the excess of
            # the wider layer and an odd tail layer feed W21 matmuls directly
            hs = hsp.tile([D, LMAX // 2], bf16, tag="hs")
            hoff = 0
            pruns = []   # (src_is_hs, col offset, width, psum col offset)
            nl = len(caps)
            t = 0
            while t + 1 < nl:
                W = int(caps[t + 1])
                o0, o1 = int(loff[i][t]), int(loff[i][t + 1])
                nc.vector.tensor_tensor(out=hs[:, hoff:hoff + W],
                                        in0=h[:, o0:o0 + W],
                                        in1=h[:, o1:o1 + W], op=Alu.add)
                pruns.append((True, hoff, W, 0))
                hoff += W
                if int(caps[t]) > W:
                    pruns.append((False, o0 + W, int(caps[t]) - W, W))
                t += 2
            if t < nl:
                pruns.append((False, int(loff[i][t]), int(caps[t]), 0))

            xc = xcp.tile([D, SL], bf16, tag="xc")
            nc.sync.dma_start(out=xc[:, :sw], in_=din["xcT"][:, lo:lo + sw])
            pa = pagg.tile([D, SL], fp32, tag="pu")
            nc.tensor.matmul(pa[:, :sw], late["Wu1"][:, :], xc[:, :sw],
                             start=True, stop=(not pruns))
            for k, (is_hs, ho, W, po_) in enumerate(pruns):
                srct = hs if is_hs else h
                nc.tensor.matmul(pa[:, po_:po_ + W], late["W21"][:, :],
                                 srct[:, ho:ho + W],
                                 start=False, stop=(k == len(pruns) - 1),
                                 skip_group_check=True)

            y1 = updp.tile([D, SL], bf16, tag="y1")
            nc.scalar.activation(y1[:, :sw], pa[:, :sw], AF.Gelu,
                                 bias=late["bu1"][:, :])
            po = pagg.tile([D, SL], fp32, tag="pu")
            nc.tensor.matmul(po[:, :sw], late["Wu2"][:, :], y1[:, :sw],
                             start=True, stop=True)
            ot = updp.tile([D, SL], bf16, tag="ot")
            with nc.allow_low_precision("bf16 output"):
                nc.vector.tensor_scalar_add(ot[:, :sw], po[:, :sw],
                                            late["bu2"][:, :])
            nc.sync.dma_start(out=outT[:, lo:lo + sw], in_=ot[:, :sw])

    nc.compile()
    return nc


# ---------------------------------------------------------------- runner
_CACHE = {}


def _in_maps(inputs, shared, per_core):
    x = np.asarray(inputs["x"], dtype=np.float32)
    edge_attr = np.asarray(inputs["edge_attr"], dtype=np.float32)
    src = np.asarray(inputs["edge_index"][0]).astype(np.int64)
    eps = float(np.asarray(inputs["eps"]).reshape(-1)[0])
    be1 = np.asarray(inputs["be1"], dtype=np.float32)
    be2 = np.asarray(inputs["be2"], dtype=np.float32)

    We1b = _bf16(inputs["We1"]).astype(np.float32)
    We2b = _bf16(inputs["We2"]).astype(np.float32)
    Wu1b = _bf16(inputs["Wu1"]).astype(np.float32)
    Wu2b = _bf16(inputs["Wu2"]).astype(np.float32)
    W21 = _bf16(We2b @ Wu1b)
    qW2 = (_gelu(be1).astype(np.float32) @ We2b).astype(np.float32)

    shared_map = {
        "We1": _bf16(inputs["We1"]),
        "W21": W21,
        "Wu1": _bf16(inputs["Wu1"]),
        "Wu2": _bf16(inputs["Wu2"]),
        "be1": be1.reshape(D, 1),
        "bu1": np.asarray(inputs["bu1"], dtype=np.float32).reshape(D, 1),
        "bu2": np.asarray(inputs["bu2"], dtype=np.float32).reshape(D, 1),
    }

    STREAM = shared["stream"]
    maps = []
    for c in range(NC):
        pc = per_core[c]
        combT = np.zeros((D, STREAM), dtype=BF16)
        eid = pc["eid"]
        combT[:, pc["slot"]] = _bf16(x[src[eid]] + edge_attr[eid]).T

        xn = x[c * NPC:(c + 1) * NPC][pc["order"]]   # node features, col order
        degc = pc["deg"][pc["order"]].astype(np.float32)
        xc = ((1.0 + eps) * xn
              + degc[:, None] * be2[None, :]
              - pc["padtot"].astype(np.float32)[:, None] * qW2[None, :])
        m = dict(shared_map)
        m.update(stream=combT, xcT=_bf16(xc.T))
        maps.append(m)
    return maps


def kernel(**inputs):
    from concourse.bass_utils import run_bass_kernel_spmd

    shared, per_core = _build_plans(inputs["edge_index"])
    key = tuple(int(l[-1]) for l in shared["loff"]) + (shared["stream"],)
    if _CACHE.get("key") != key:
        _CACHE["nc"] = _build_bass(shared)
        _CACHE["key"] = key
    nc = _CACHE["nc"]
    maps = _in_maps(inputs, shared, per_core)
    res = run_bass_kernel_spmd(nc, maps, core_ids=list(range(NC)))
    _CACHE["last_results"] = res
    out = np.zeros((N, D), dtype=np.float32)
    for c in range(NC):
        col_of = per_core[c]["col_of"]
        out[c * NPC:(c + 1) * NPC] = \
            res.results[c]["outT"].astype(np.float32)[:, col_of].T
    return out


# revision 34
# speedup vs baseline: 1.0333x; 1.0178x over previous
"""Trainium2 Bass kernel for nn_DenseGINEConv (GNN message passing).

  out = MLP_u((1+eps)*x + segsum_dst(MLP_e(x[src] + edge_attr)))

Strategy (8 NeuronCores, nodes sharded by dst, 6250/core), v2
("all-matmul segment sum"):

- Nodes are sorted by degree (desc) per core; edges are packed in *layer*
  order per 512-node slice: layer j holds edge j of every node with more
  than j edges.  Because nodes are degree-sorted, layer j occupies node
  columns [0, m_j) -- a contiguous, aligned run.  Each node's slot count is
  padded to even, so layers come in equal-width pairs.  Total stream is
  ~79k slots/core vs 114.7k for 16-slot group padding.
- Edge MLP layer 2 and update MLP layer 1 are composed host-side
  (W21 = We2 @ Wu1), so the per-edge pipeline is:
     h = GELU(We1^T xg + be1)           (tensor + scalar)
     hs = h[layer 2t] + h[layer 2t+1]   (vector, bf16 2x mode)
     psum_slice += W21^T @ hs[...]      (tensor; PSUM accumulation IS the
                                         segment sum -- no scatter, no
                                         tensor_reduce anywhere)
- All node-constant terms are folded host-side into one tensor:
     xc = (1+eps)x + deg*be2 - padtot*(GELU(be1)@We2)
  consumed by a single Wu1 matmul into the same PSUM accumulation.
  Then: y1 = GELU(psum + bu1); out = Wu2^T y1 + bu2.
- The per-core layer widths are maxed across cores so one Bass program
  (compiled at first kernel() call, from the actual input's degree
  profile) serves all 8 cores SPMD; narrower cores ride zero-pad columns
  whose GELU(be1) contribution is exactly corrected via padtot.
"""

import math
import os
import sys
from contextlib import ExitStack

import numpy as np
import ml_dtypes

# concourse (Bass) lives in the trn_rl repo; make kernel.py self-contained
for _p in ("/opt/trn_rl_repo",):
    if os.path.isdir(_p) and _p not in sys.path:
        sys.path.insert(0, _p)

# ---------------------------------------------------------------- constants
N = 50000
E = 600000
D = 128
NC = 8
NPC = N // NC                 # 6250 nodes/core
SL = 512                      # node-slice width (PSUM bank)
NSLICE = (NPC + SL - 1) // SL
GTILE = 1536                  # edge GELU tile width (3 PSUM banks)

BF16 = ml_dtypes.bfloat16


def _gelu(z):
    z = np.asarray(z, dtype=np.float64)
    return 0.5 * z * (1.0 + np.vectorize(math.erf)(z / math.sqrt(2.0)))


def _bf16(a):
    return np.asarray(a).astype(BF16)


# ---------------------------------------------------------------- host plan
def _build_plans(edge_index):
    """Returns (shared, per_core).

    shared: M[i] = layer widths (even count, non-increasing) per slice,
            loff[i] = col offset of each layer inside the slice chunk,
            choff[i] = chunk offset in the stream, stream = total cols.
    per_core: col_of (node->column), deg, padtot (per column), and the
            stream slot index of every edge (by original edge id).
    """
    src = np.asarray(edge_index[0]).astype(np.int64)
    dst = np.asarray(edge_index[1]).astype(np.int64)

    core_of = dst // NPC
    dst_local = dst - core_of * NPC

    per_core = []
    pads_all = []
    for c in range(NC):
        msk = core_of == c
        cloc = dst_local[msk]
        deg = np.bincount(cloc, minlength=NPC).astype(np.int64)
        order = np.argsort(-deg, kind="stable")      # node ids by deg desc
        col_of = np.empty(NPC, dtype=np.int64)
        col_of[order] = np.arange(NPC)
        per_core.append(dict(col_of=col_of, deg=deg, order=order,
                             eid=np.nonzero(msk)[0], cloc=cloc))
        pads_all.append(deg[order])                  # exact slots per column

    # shared layer caps per slice: M_j(i) = max over cores of
    # #{cols in slice with padded slots > j}
    M, loff, choff, = [], [], []
    off = 0
    for i in range(NSLICE):
        lo, hi = i * SL, min((i + 1) * SL, NPC)
        caps = None
        for c in range(NC):
            p = pads_all[c][lo:hi]
            pm = int(p.max()) if len(p) else 0
            cnt = np.bincount(p, minlength=pm + 1)
            mj = len(p) - np.cumsum(cnt)[:-1]        # m_j = #{p > j}, j=0..pm-1
            mj = mj[mj > 0]
            if caps is None:
                caps = mj
            else:
                n = max(len(caps), len(mj))
                a = np.zeros(n, dtype=np.int64); a[:len(caps)] = caps
                b = np.zeros(n, dtype=np.int64); b[:len(mj)] = mj
                caps = np.maximum(a, b)
        if caps is None or len(caps) == 0:
            caps = np.zeros(0, dtype=np.int64)
        M.append(caps)
        lo_ = np.zeros(len(caps) + 1, dtype=np.int64)
        np.cumsum(caps, out=lo_[1:])
        loff.append(lo_)
        choff.append(off)
        off += int(lo_[-1])
    stream = off

    # per-core: slot index for each edge + padtot per column
    for c in range(NC):
        pc = per_core[c]
        cloc = pc["cloc"]
        col = pc["col_of"][cloc]                     # column of each edge
        sl = col // SL
        pos = col - sl * SL
        # rank of edge within its node (edges arbitrary order)
        o = np.argsort(col, kind="stable")
        col_s = col[o]
        starts = np.searchsorted(col_s, np.arange(NPC))
        rank = np.empty(len(col), dtype=np.int64)
        rank[o] = np.arange(len(col)) - starts[col_s]
        # chunk offset + layer offset + pos
        ch = np.asarray(choff, dtype=np.int64)[sl]
        lof = np.empty(len(col), dtype=np.int64)
        for i in range(NSLICE):
            m = sl == i
            if m.any():
                lof[m] = loff[i][rank[m]]
        slot = ch + lof + pos
        pc["slot"] = slot

        # padtot per column: #layers covering the column minus deg
        padtot = np.zeros(NPC, dtype=np.int64)
        for i in range(NSLICE):
            lo, hi = i * SL, min((i + 1) * SL, NPC)
            w = hi - lo
            p = np.arange(w)
            cover = (M[i][None, :] > p[:, None]).sum(axis=1)
            padtot[lo:hi] = cover
        padtot -= pc["deg"][pc["order"]]
        assert padtot.min() >= 0
        pc["padtot"] = padtot
    shared = dict(M=M, loff=loff, choff=choff, stream=stream)
    return shared, per_core


# ---------------------------------------------------------------- bass build
def _build_bass(shared):
    import concourse.mybir as mybir
    from concourse import bacc
    from concourse._compat import get_trn_type
    from concourse.tile import TileContext

    fp32 = mybir.dt.float32
    bf16 = mybir.dt.bfloat16
    AF = mybir.ActivationFunctionType
    Alu = mybir.AluOpType

    STREAM = shared["stream"]
    M = shared["M"]
    loff = shared["loff"]
    choff = shared["choff"]
    LMAX = max(int(l[-1]) for l in loff)
    LMAX = (LMAX + 511) // 512 * 512

    nc = bacc.Bacc(get_trn_type() or "TRN2")

    din = {}
    for name, shape, dt in [
        ("stream", [D, STREAM], bf16),
        ("xcT", [D, NPC], bf16),
        ("We1", [D, D], bf16),
        ("W21", [D, D], bf16),
        ("Wu1", [D, D], bf16),
        ("Wu2", [D, D], bf16),
        ("be1", [D, 1], fp32),
        ("bu1", [D, 1], fp32),
        ("bu2", [D, 1], fp32),
    ]:
        din[name] = nc.declare_dram_parameter(name, shape, dt, isOutput=False)
    outT = nc.declare_dram_parameter("outT", [D, NPC], bf16, isOutput=True)

    with TileContext(nc) as tc, ExitStack() as ctx:
        consts = ctx.enter_context(tc.tile_pool(name="consts", bufs=1))
        xgp = ctx.enter_context(tc.tile_pool(name="xg", bufs=12))
        hp = ctx.enter_context(tc.tile_pool(name="h", bufs=3))
        hsp = ctx.enter_context(tc.tile_pool(name="hs", bufs=3))
        updp = ctx.enter_context(tc.tile_pool(name="upd", bufs=6))
        xcp = ctx.enter_context(tc.tile_pool(name="xc", bufs=4))
        pse = ctx.enter_context(tc.tile_pool(name="pse", bufs=2, space="PSUM"))
        pagg = ctx.enter_context(tc.tile_pool(name="pagg", bufs=2, space="PSUM"))

        def load(name, shape, dt):
            # consts issue from the (startup-idle) scalar HWDGE queue so the
            # sync queue starts streaming edge units immediately
            t = consts.tile(shape, dt, tag=name)
            nc.scalar.dma_start(out=t[:, :], in_=din[name][:, :])
            return t

        # warm the Gelu activation table while DMAs fill (saves ~1.3us on
        # the scalar critical path)
        warm = consts.tile([D, 1], fp32, tag="warm")
        nc.vector.memset(warm[:, :], 0.0)
        nc.scalar.activation(warm[:, :], warm[:, :], AF.Gelu, bias=0.0)

        We1 = load("We1", [D, D], bf16)
        be1 = load("be1", [D, 1], fp32)
        late = {name: load(name, shape, dt)
                for name, shape, dt in [("W21", [D, D], bf16),
                                        ("Wu1", [D, D], bf16),
                                        ("Wu2", [D, D], bf16),
                                        ("bu1", [D, 1], fp32),
                                        ("bu2", [D, 1], fp32)]}

        for i in range(NSLICE):
            lo = i * SL
            sw = min(SL, NPC - lo)
            L = int(loff[i][-1])
            caps = M[i]

            h = hp.tile([D, LMAX], bf16, tag="h")
            nt = (L + GTILE - 1) // GTILE
            for t in range(nt):
                t0 = t * GTILE
                w = min(GTILE, L - t0)
                xg = xgp.tile([D, GTILE], bf16, tag="xg")
                nc.sync.dma_start(
                    out=xg[:, :w],
                    in_=din["stream"][:, choff[i] + t0:choff[i] + t0 + w])
                ps = pse.tile([D, GTILE], fp32, tag="pse")
                for q0 in range(0, w, 512):
                    q1 = min(q0 + 512, w)
                    nc.tensor.matmul(ps[:, q0:q1], We1[:, :],
                                     xg[:, q0:q1],
                                     start=True, stop=True)
                nc.scalar.activation(h[:, t0:t0 + w], ps[:, :w], AF.Gelu,
                                     bias=be1[:, :])

            # pair layers for one bf16 add round (2x DVE mode); the excess of
            # the wider layer and an odd tail layer feed W21 matmuls directly
            hs = hsp.tile([D, LMAX // 2], bf16, tag="hs")
            hoff = 0
            pruns = []   # (src_is_hs, col offset, width, psum col offset)
            nl = len(caps)
            t = 0
            while t + 1 < nl:
                W = int(caps[t + 1])
                o0, o1 = int(loff[i][t]), int(loff[i][t + 1])
                nc.vector.tensor_tensor(out=hs[:, hoff:hoff + W],
                                        in0=h[:, o0:o0 + W],
                                        in1=h[:, o1:o1 + W], op=Alu.add)
                pruns.append((True, hoff, W, 0))
                hoff += W
                if int(caps[t]) > W:
                    pruns.append((False, o0 + W, int(caps[t]) - W, W))
                t += 2
            if t < nl:
                pruns.append((False, int(loff[i][t]), int(caps[t]), 0))

            xc = xcp.tile([D, SL], bf16, tag="xc")
            nc.sync.dma_start(out=xc[:, :sw], in_=din["xcT"][:, lo:lo + sw])
            pa = pagg.tile([D, SL], fp32, tag="pu")
            nc.tensor.matmul(pa[:, :sw], late["Wu1"][:, :], xc[:, :sw],
                             start=True, stop=(not pruns))
            for k, (is_hs, ho, W, po_) in enumerate(pruns):
                srct = hs if is_hs else h
                nc.tensor.matmul(pa[:, po_:po_ + W], late["W21"][:, :],
                                 srct[:, ho:ho + W],
                                 start=False, stop=(k == len(pruns) - 1),
                                 skip_group_check=True)

            y1 = updp.tile([D, SL], bf16, tag="y1")
            nc.scalar.activation(y1[:, :sw], pa[:, :sw], AF.Gelu,
                                 bias=late["bu1"][:, :])
            po = pagg.tile([D, SL], fp32, tag="pu")
            nc.tensor.matmul(po[:, :sw], late["Wu2"][:, :], y1[:, :sw],
                             start=True, stop=True)
            ot = updp.tile([D, SL], bf16, tag="ot")
            with nc.allow_low_precision("bf16 output"):
                nc.vector.tensor_scalar_add(ot[:, :sw], po[:, :sw],
                                            late["bu2"][:, :])
            nc.sync.dma_start(out=outT[:, lo:lo + sw], in_=ot[:, :sw])

    nc.compile()
    return nc


# ---------------------------------------------------------------- runner
_CACHE = {}


def _in_maps(inputs, shared, per_core):
    x = np.asarray(inputs["x"], dtype=np.float32)
    edge_attr = np.asarray(inputs["edge_attr"], dtype=np.float32)
    src = np.asarray(inputs["edge_index"][0]).astype(np.int64)
    eps = float(np.asarray(inputs["eps"]).reshape(-1)[0])
    be1 = np.asarray(inputs["be1"], dtype=np.float32)
    be2 = np.asarray(inputs["be2"], dtype=np.float32)

    We1b = _bf16(inputs["We1"]).astype(np.float32)
    We2b = _bf16(inputs["We2"]).astype(np.float32)
    Wu1b = _bf16(inputs["Wu1"]).astype(np.float32)
    Wu2b = _bf16(inputs["Wu2"]).astype(np.float32)
    W21 = _bf16(We2b @ Wu1b)
    qW2 = (_gelu(be1).astype(np.float32) @ We2b).astype(np.float32)

    shared_map = {
        "We1": _bf16(inputs["We1"]),
        "W21": W21,
        "Wu1": _bf16(inputs["Wu1"]),
        "Wu2": _bf16(inputs["Wu2"]),
        "be1": be1.reshape(D, 1),
        "bu1": np.asarray(inputs["bu1"], dtype=np.float32).reshape(D, 1),
        "bu2": np.asarray(inputs["bu2"], dtype=np.float32).reshape(D, 1),
    }

    STREAM = shared["stream"]
    maps = []
    for c in range(NC):
        pc = per_core[c]
        combT = np.zeros((D, STREAM), dtype=BF16)
        eid = pc["eid"]
        combT[:, pc["slot"]] = _bf16(x[src[eid]] + edge_attr[eid]).T

        xn = x[c * NPC:(c + 1) * NPC][pc["order"]]   # node features, col order
        degc = pc["deg"][pc["order"]].astype(np.float32)
        xc = ((1.0 + eps) * xn
              + degc[:, None] * be2[None, :]
              - pc["padtot"].astype(np.float32)[:, None] * qW2[None, :])
        m = dict(shared_map)
        m.update(stream=combT, xcT=_bf16(xc.T))
        maps.append(m)
    return maps


def kernel(**inputs):
    from concourse.bass_utils import run_bass_kernel_spmd

    shared, per_core = _build_plans(inputs["edge_index"])
    key = tuple(int(l[-1]) for l in shared["loff"]) + (shared["stream"],)
    if _CACHE.get("key") != key:
        _CACHE["nc"] = _build_bass(shared)
        _CACHE["key"] = key
    nc = _CACHE["nc"]
    maps = _in_maps(inputs, shared, per_core)
    res = run_bass_kernel_spmd(nc, maps, core_ids=list(range(NC)))
    _CACHE["last_results"] = res
    out = np.zeros((N, D), dtype=np.float32)
    for c in range(NC):
        col_of = per_core[c]["col_of"]
        out[c * NPC:(c + 1) * NPC] = \
            res.results[c]["outT"].astype(np.float32)[:, col_of].T
    return out
